# revision 22
# baseline (speedup 1.0000x reference)
"""Trainium2 Bass kernel: autoregressive 2-layer GRU decoder (13 steps).

Strategy (per core, batch-sharded 1024/8 = 128):
- Everything lives transposed on-chip: [feature -> partitions, batch -> free].
  GRU biases become per-partition scalars (fused into ScalarE activation /
  DVE scalar_tensor_tensor), and no transposes are needed in the recurrence.
- The 13 autoregressive decode chains are batched along a diagonal wavefront:
  at wavefront step tau, chain k processes window position j = tau - k.  All
  active chains at a given tau read the SAME window element (x_tau or
  y_{tau-13}); it is replicated across the chain block once per tau and its
  projection rides the gate matmul accumulation as extra K-chunks.
- The three big weight matrices (W_hh0, W_ih1, W_hh1; 97% of MACs) are
  GPTQ-quantized to fp8 e4m3 (x4096, clipped to TRN's +-240) against the
  actual h0/h1 activation covariance, and their matmuls run in DoubleRow
  perf mode (K=256/instruction, 2 MACs/cell/cycle).  Hidden states are
  stored as e4m3 at natural scale.  The 2^12 weight scale is folded into
  the ScalarE activation `scale` and pre-scaled gate biases.  W_ih0 (also
  pre-scaled) and W_out stay bf16.
"""
import numpy as np

B, T, Y, H = 1024, 13, 188, 512
H3 = 3 * H
NCORE = 8
BS = B // NCORE          # 128 batch rows per core
HC = H // 128            # 4 H chunks
GC = H3 // 128           # 12 gate chunks
Y1 = Y - 128             # 60 rows in second Y chunk
CPB = 4                  # chains per N-block (4*128 = 512 cols per matmul)

WSCALE = 4096.0          # fp8 weight scale (power of 2)
SINV = 1.0 / WSCALE

GATE_BUFS = 8
TMP_BUFS = 8
EP_BUFS = 8
ER_BUFS = 3
PSA_BUFS = 6

# bf16 wall: W_ih0 (2 K-chunks, pre-scaled x4096) + W_out (4 K-chunks) + I128
OFF_WIH0A = 0
OFF_WIH0B = H3
OFF_WOUT = 2 * H3
OFF_ID = OFF_WOUT + 4 * Y
NWALL_BF = OFF_ID + 128
# fp8 wall [128, 12, H3]: W_hh0 kc 0-3, W_ih1 kc 4-7, W_hh1 kc 8-11
W8_HH0 = 0
W8_IH1 = 4
W8_HH1 = 8

_CACHE = {}
_last_in_maps = None


def _build():
    from contextlib import ExitStack
    import concourse.tile as tile
    from concourse import bacc, mybir

    F32 = mybir.dt.float32
    AF = mybir.ActivationFunctionType
    OP = mybir.AluOpType
    DR = mybir.MatmulPerfMode.DoubleRow

    BF16 = mybir.dt.bfloat16
    F8 = mybir.dt.float8e4
    nc = bacc.Bacc("TRN2", target_bir_lowering=False, debug=False)
    xt = nc.declare_dram_parameter("xt", [Y, T, BS], F32, isOutput=False)
    wall = nc.declare_dram_parameter("wall", [128, NWALL_BF], BF16, isOutput=False)
    w8 = nc.declare_dram_parameter("w8", [128, 12, H3], F8, isOutput=False)
    bi = nc.declare_dram_parameter("bi", [128, 34 + 128], F32, isOutput=False)
    out = nc.declare_dram_parameter("out", [T, Y, BS], F32, isOutput=True)

    with tile.TileContext(nc) as tc, ExitStack() as ctx:
        wp = ctx.enter_context(tc.tile_pool(name="w", bufs=1))
        hp = ctx.enter_context(tc.tile_pool(name="h", bufs=1))
        ep = ctx.enter_context(tc.tile_pool(name="ep", bufs=EP_BUFS))
        ebp = ctx.enter_context(tc.tile_pool(name="ebp", bufs=ER_BUFS))
        xgp = ctx.enter_context(tc.tile_pool(name="xgp", bufs=2))
        gp = ctx.enter_context(tc.tile_pool(name="g", bufs=GATE_BUFS))
        psA = ctx.enter_context(tc.tile_pool(name="psA", bufs=PSA_BUFS, space="PSUM"))
        psO = ctx.enter_context(tc.tile_pool(name="psO", bufs=2, space="PSUM"))

        # ---------------- weights / constants ----------------
        wall_t = wp.tile([128, NWALL_BF], BF16, tag="wall", name="wall")
        nc.sync.dma_start(wall_t[:, :], wall[:, :])
        w8_t = wp.tile([128, 12, H3], F8, tag="w8", name="w8")
        nc.sync.dma_start(w8_t[:, :, :], w8[:, :, :])
        bi_t = wp.tile([128, 34 + 128], F32, tag="bi", name="bi")
        nc.sync.dma_start(bi_t[:, :], bi[:, :])
        wih0_t = [wall_t[:, OFF_WIH0A:OFF_WIH0A + H3],
                  wall_t[:Y1, OFF_WIH0B:OFF_WIH0B + H3]]
        wout_t = [wall_t[:, OFF_WOUT + kc * Y:OFF_WOUT + (kc + 1) * Y]
                  for kc in range(HC)]
        ident_t = wall_t[:, OFF_ID:OFF_ID + 128]
        bias_t = bi_t[:, 0:34]

        # ---------------- persistent state ----------------
        # h stored as fp8 e4m3 (natural scale); a chain's first step (j=0,
        # h=0) skips the recurrent matmuls and uses the h = (1-z)*n update
        hall = hp.tile([128, 2, HC, T, BS], F8, tag="hall", name="hall")
        h0t = hall[:, 0]
        h1t = hall[:, 1]

        # ---------------- helpers ----------------
        def gru_phase(layer, tau, k0, k1, xg0, hlast=None, blocks=None):
            """One wavefront step of a GRU layer over chains k0..k1 inclusive.

            layer 0: shared input projection xg0 [128, GC, BS] (bf16,
            x WSCALE) is broadcast-added across chains on DVE; PSUM holds
            only the recurrent projection.
            layer 1: input projection (from h0) + recurrent projection are
            both accumulated in PSUM.
            The freshest chain (window position j=0) has h=0: its recurrent
            matmuls are skipped and its update is h = (1-z)*n.
            Recurrent/input matmuls are fp8 DoubleRow; psums are x WSCALE.
            For layer 1, the finishing chain's (j=12) h update is written to
            `hlast` in bf16 instead of the fp8 state (feeds W_out directly).
            """
            w8base = W8_HH0 if layer == 0 else W8_HH1
            h = h0t if layer == 0 else h1t
            brz_c = 0 if layer == 0 else 16
            bhn_c = 8 if layer == 0 else 24
            bin_c = 12 if layer == 0 else 28
            fresh_k = tau if layer == 0 else tau - 1   # chain at j=0, if any
            A = k1 - k0 + 1
            if blocks is None:
                blocks = range(0, A, CPB)
            for b0 in blocks:
                ch = min(CPB, A - b0)
                ks = k0 + b0
                # chains with j>0 in this block (fresh chain is always k1,
                # the last chain of the last block)
                cr = ch - 1 if ks + ch - 1 == fresh_k else ch
                # layer-1 chain at its last position, written to hlast
                fin = layer == 1 and hlast is not None and b0 == 0

                def in_proj(ps, m, first, stop):
                    # layer-1 input projection K-pairs into psum ps
                    for j in range(2):
                        nc.tensor.matmul(
                            ps,
                            w8_t[:, W8_IH1 + 2 * j:W8_IH1 + 2 * j + 2,
                                 m * 128:(m + 1) * 128],
                            h0t[:, 2 * j:2 * j + 2, ks:ks + ch, :],
                            start=(first and j == 0),
                            stop=(stop and j == 1), perf_mode=DR)

                def rec_proj(ps, m, first, stop):
                    # recurrent K-pairs, only over the cr non-fresh chains
                    for j in range(2):
                        nc.tensor.matmul(
                            ps,
                            w8_t[:, w8base + 2 * j:w8base + 2 * j + 2,
                                 m * 128:(m + 1) * 128],
                            h[:, 2 * j:2 * j + 2, ks:ks + cr, :],
                            start=(first and j == 0),
                            stop=(stop and j == 1), perf_mode=DR)

                def bc(m, n):
                    return xg0[:, m:m + 1, :].broadcast_to((128, n, BS))

                rzt = {}
                nts = []
                for m in range(8):  # r (0-3) and z (4-7) gates
                    g = gp.tile([128, CPB, BS], BF16, tag="rz", name="rz",
                                bufs=GATE_BUFS)
                    if layer == 0:
                        # psum = rec (fp8 DoubleRow) + xg0 broadcast via an
                        # identity bf16 matmul; fresh chain gets xg0 only
                        # (overwrite where has_written is unset)
                        ps = psA.tile([128, CPB, BS], F32, tag="ps",
                                      name="ps")
                        if cr > 0:
                            rec_proj(ps[:, :cr, :], m, True, False)
                        nc.tensor.matmul(ps[:, :ch, :], ident_t, bc(m, ch),
                                         start=(cr == 0), stop=True,
                                         skip_group_check=True)
                        nc.scalar.activation(g[:, :ch, :], ps[:, :ch, :],
                                             AF.Sigmoid,
                                             bias=bias_t[:, brz_c + m:brz_c + m + 1],
                                             scale=SINV)
                    else:
                        ps = psA.tile([128, CPB, BS], F32, tag="ps", name="ps")
                        if cr > 0:
                            rec_proj(ps[:, :cr, :], m, True, False)
                        in_proj(ps[:, :ch, :], m, cr == 0, True)
                        nc.scalar.activation(g[:, :ch, :], ps[:, :ch, :],
                                             AF.Sigmoid,
                                             bias=bias_t[:, brz_c + m:brz_c + m + 1],
                                             scale=SINV)
                    rzt[m] = g
                for i in range(4):  # n gate
                    m = 8 + i
                    tmp = gp.tile([128, CPB, BS], BF16, tag="tmp", name="tmp",
                                  bufs=TMP_BUFS)
                    if cr > 0:
                        ghn = psA.tile([128, CPB, BS], F32, tag="ps",
                                       name="ghn")
                        rec_proj(ghn[:, :cr, :], m, True, True)
                        nc.vector.scalar_tensor_tensor(
                            tmp[:, :cr, :], ghn[:, :cr, :],
                            bias_t[:, bhn_c + i:bhn_c + i + 1],
                            rzt[i][:, :cr, :], OP.add, OP.mult)
                    if cr < ch:  # fresh chain: gh = 0, tmp = b_hn * r
                        nc.vector.tensor_scalar(
                            tmp[:, cr, :], rzt[i][:, cr, :],
                            bias_t[:, bhn_c + i:bhn_c + i + 1], None, OP.mult)
                    nt = gp.tile([128, CPB, BS], BF16, tag="n", name="n",
                                 bufs=GATE_BUFS)
                    if layer == 0:
                        nc.vector.tensor_add(nt[:, :ch, :], tmp[:, :ch, :],
                                             bc(m, ch))
                    else:
                        xgn = psA.tile([128, CPB, BS], F32, tag="ps",
                                       name="xgn")
                        in_proj(xgn[:, :ch, :], m, True, True)
                        nc.vector.tensor_add(nt[:, :ch, :], tmp[:, :ch, :],
                                             xgn[:, :ch, :])
                    nc.scalar.activation(nt[:, :ch, :], nt[:, :ch, :], AF.Tanh,
                                         bias=bias_t[:, bin_c + i:bin_c + i + 1],
                                         scale=SINV)
                    nts.append(nt)
                for i in range(4):  # h <- n + z*(h - n);  fresh: h <- n - z*n
                    z = rzt[4 + i]
                    d = gp.tile([128, CPB, BS], BF16, tag="tmp", name="d",
                                bufs=TMP_BUFS)
                    if cr > 0:
                        hsl = h[:, i, ks:ks + cr, :]
                        nc.vector.tensor_sub(d[:, :cr, :], hsl,
                                             nts[i][:, :cr, :])
                        nc.vector.tensor_mul(d[:, :cr, :], d[:, :cr, :],
                                             z[:, :cr, :])
                        if fin:
                            # finishing chain -> bf16 staging for W_out
                            nc.vector.tensor_add(hlast[:, i, :], d[:, 0, :],
                                                 nts[i][:, 0, :])
                            if cr > 1:
                                nc.vector.tensor_add(
                                    h[:, i, ks + 1:ks + cr, :],
                                    d[:, 1:cr, :], nts[i][:, 1:cr, :])
                        else:
                            nc.vector.tensor_add(hsl, d[:, :cr, :],
                                                 nts[i][:, :cr, :])
                    if cr < ch:
                        nc.vector.tensor_mul(d[:, cr, :], z[:, cr, :],
                                             nts[i][:, cr, :])
                        nc.vector.tensor_sub(h[:, i, ks + cr, :],
                                             nts[i][:, cr, :], d[:, cr, :])

        def eproj(e_tile, ebf=None):
            """Shared input projection: e [Y, BS] -> xg0 [3H(12 chunks), BS].

            Casts the f32 window element to bf16 (unless out_phase already
            produced it), then 24 N=128 matmuls.
            W_ih0 is pre-scaled by WSCALE, so xg0 is in the scaled domain.
            """
            if ebf is None:
                ebf = ebp.tile([128, 2, BS], BF16, tag="ebf", name="ebf",
                               bufs=ER_BUFS)
                nc.vector.tensor_copy(ebf[:, 0, :], e_tile[:, 0, :])
                nc.vector.tensor_copy(ebf[:Y1, 1, :], e_tile[:Y1, 1, :])
            xg0 = xgp.tile([128, GC, BS], BF16, tag="xg0", name="xg0")
            for third in range(3):
                pe = psO.tile([128, 4, BS], F32, tag="pe", name="pe")
                for mi in range(4):
                    m = third * 4 + mi
                    nc.tensor.matmul(pe[:, mi, :],
                                     wih0_t[0][:, m * 128:(m + 1) * 128],
                                     ebf[:, 0, :], start=True, stop=False)
                    nc.tensor.matmul(pe[:, mi, :],
                                     wih0_t[1][:, m * 128:(m + 1) * 128],
                                     ebf[:Y1, 1, :], start=False, stop=True)
                if third == 1:
                    nc.vector.tensor_copy(
                        xg0[:, third * 4:(third + 1) * 4, :], pe[:, :, :])
                else:
                    nc.scalar.copy(xg0[:, third * 4:(third + 1) * 4, :],
                                   pe[:, :, :])
            return xg0

        def out_phase(tau, elems_bf, hlast):
            """Emit y_{tau-13} = W_out @ relu(h1) + b_out + residual.

            The bf16 residual (previous window element) is accumulated into
            the W_out psum via an identity matmul, and b_out rides the
            ScalarE copy bias, so no DVE op sits on the feedback path."""
            c = tau - 13
            rl = gp.tile([128, HC, BS], BF16, tag="rl", name="rl", bufs=2)
            nc.scalar.activation(rl[:, :, :], hlast[:, :, :], AF.Relu)
            po = psO.tile([128, 4, BS], F32, tag="pe", name="po")[:, :2, :]
            ebr = elems_bf[c + 12]
            for kc in range(HC):
                nc.tensor.matmul(po[:, 0, :], wout_t[kc][:, 0:128],
                                 rl[:, kc, :],
                                 start=(kc == 0), stop=False)
            nc.tensor.matmul(po[:, 0, :], ident_t, ebr[:, 0, :],
                             start=False, stop=True, skip_group_check=True)
            for kc in range(HC):
                nc.tensor.matmul(po[:Y1, 1, :], wout_t[kc][:, 128:Y],
                                 rl[:, kc, :],
                                 start=(kc == 0), stop=False)
            nc.tensor.matmul(po[:Y1, 1, :], ident_t[:Y1, 0:Y1],
                             ebr[:Y1, 1, :],
                             start=False, stop=True, skip_group_check=True)
            ebf = ebp.tile([128, 2, BS], BF16, tag="ebf", name="ebf",
                           bufs=ER_BUFS)
            # bf16 copy first: it heads the y -> xg0 -> L0 feedback path
            nc.scalar.add(ebf[:, 0, :], po[:, 0, :], bias_t[:, 32:33])
            nc.scalar.add(ebf[:Y1, 1, :], po[:Y1, 1, :], bias_t[:Y1, 33:34])
            y = ep.tile([128, 2, BS], F32, tag="e", name="y", bufs=EP_BUFS)
            nc.scalar.add(y[:, 0, :], po[:, 0, :], bias_t[:, 32:33])
            nc.scalar.add(y[:Y1, 1, :], po[:Y1, 1, :], bias_t[:Y1, 33:34])
            # output stays transposed [feature, batch]; host un-transposes
            nc.sync.dma_start(out[c, 0:128, :], y[:, 0, :])
            nc.sync.dma_start(out[c, 128:Y, :], y[:Y1, 1, :])
            return ebf

        # ---------------- wavefront ----------------
        # Issue order per tau: feedback-critical prefix first (L1 block 0 ->
        # out -> eproj -> L0 block 0), then bulk blocks interleaved.  The
        # engine queues are FIFO, so this keeps the autoregressive y ->
        # xg0 -> L0 -> L1 -> y loop from queuing behind bulk gate work.
        elems = {}
        elems_bf = {}
        for tau in range(26):
            if tau <= 12:
                et = ep.tile([128, 2, BS], F32, tag="e", name="e",
                             bufs=EP_BUFS)
                nc.sync.dma_start(et[:, 0, :], xt[0:128, tau, :])
                nc.sync.dma_start(et[:Y1, 1, :], xt[128:Y, tau, :])
                elems[tau] = et
            k0_1, k1_1 = max(0, tau - 13), min(T - 1, tau - 1)
            has1 = k0_1 <= k1_1
            A1 = k1_1 - k0_1 + 1 if has1 else 0
            b1list = list(range(0, A1, CPB))
            k0_0, k1_0 = max(0, tau - 12), min(T - 1, tau)
            has0 = tau <= 24
            A0 = k1_0 - k0_0 + 1 if has0 else 0
            b0list = list(range(0, A0, CPB))
            hlast = None
            if tau >= 13:
                hlast = gp.tile([128, HC, BS], BF16, tag="hlast",
                                name="hlast", bufs=2)
            # --- critical prefix ---
            if has1:
                gru_phase(1, tau, k0_1, k1_1, None, hlast=hlast,
                          blocks=b1list[:1])
            ebf = None
            if tau >= 13:
                ebf = out_phase(tau, elems_bf, hlast)
                elems_bf[tau] = ebf
            xg0 = None
            if has0:
                if ebf is None:
                    ebf = ebp.tile([128, 2, BS], BF16, tag="ebf", name="ebf",
                                   bufs=ER_BUFS)
                    nc.vector.tensor_copy(ebf[:, 0, :], elems[tau][:, 0, :])
                    nc.vector.tensor_copy(ebf[:Y1, 1, :],
                                          elems[tau][:Y1, 1, :])
                    elems_bf[tau] = ebf
                xg0 = eproj(elems.get(tau), ebf)
                gru_phase(0, tau, k0_0, k1_0, xg0, blocks=b0list[:1])
            # --- bulk blocks ---
            if has1 and len(b1list) > 1:
                gru_phase(1, tau, k0_1, k1_1, None, hlast=hlast,
                          blocks=b1list[1:])
            if has0 and len(b0list) > 1:
                gru_phase(0, tau, k0_0, k1_0, xg0, blocks=b0list[1:])

    nc.finalize()
    return nc


def _gru_layer_np(x, W_ih, W_hh, b_ih, b_hh):
    """Full-precision numpy GRU layer for GPTQ calibration."""
    Bc, Tc = x.shape[0], x.shape[1]
    xg = np.einsum('btd,gd->btg', x, W_ih) + b_ih
    h = np.zeros((Bc, H), np.float32)
    hs = []
    for t in range(Tc):
        gh = h @ W_hh.T + b_hh
        r = 1.0 / (1.0 + np.exp(-(xg[:, t, :H] + gh[:, :H])))
        z = 1.0 / (1.0 + np.exp(-(xg[:, t, H:2 * H] + gh[:, H:2 * H])))
        n = np.tanh(xg[:, t, 2 * H:] + r * gh[:, 2 * H:])
        h = (1.0 - z) * n + z * h
        hs.append(h)
    return np.stack(hs, axis=1)


def _q8_grid(a):
    """Round f32 to the e4m3*WSCALE grid (TRN clip at +-240), back to f32."""
    import ml_dtypes
    x = np.clip(np.asarray(a, np.float32) * WSCALE, -240.0, 240.0)
    return np.asarray(x, ml_dtypes.float8_e4m3).astype(np.float32) / WSCALE


def _gptq(W, Hmat, damp=0.01):
    """GPTQ-quantize W [rows, K] to the e4m3 grid given Hessian E[x x^T]."""
    Wf = np.array(W, np.float64)
    K = Wf.shape[1]
    Hd = Hmat + np.eye(K) * damp * np.mean(np.diag(Hmat))
    perm = np.argsort(-np.diag(Hd))
    inv_perm = np.argsort(perm)
    Hd = Hd[perm][:, perm]
    Wp = Wf[:, perm]
    Hinv = np.linalg.inv(Hd)
    U = np.linalg.cholesky(Hinv).T        # upper, Hinv = U^T U
    Q = np.zeros_like(Wp)
    for j in range(K):
        w = Wp[:, j]
        q = _q8_grid(w.astype(np.float32)).astype(np.float64)
        Q[:, j] = q
        err = (w - q) / U[j, j]
        if j + 1 < K:
            Wp[:, j + 1:] -= np.outer(err, U[j, j + 1:])
    return Q[:, inv_perm].astype(np.float32)


def _prep_in_maps(inputs):
    import ml_dtypes
    x = np.asarray(inputs["x"], np.float32)
    f = lambda k: np.asarray(inputs[k], np.float32)
    W_ih0, W_hh0 = f("W_ih0"), f("W_hh0")
    W_ih1, W_hh1 = f("W_ih1"), f("W_hh1")
    W_out = f("W_out")
    b_ih0, b_hh0 = f("b_ih0"), f("b_hh0")
    b_ih1, b_hh1 = f("b_ih1"), f("b_hh1")
    b_out = f("b_out")

    # ---- GPTQ calibration on actual activation trajectories ----
    xs = x[:512]
    h0s = _gru_layer_np(xs, W_ih0, W_hh0, b_ih0, b_hh0)
    h1s = _gru_layer_np(h0s, W_ih1, W_hh1, b_ih1, b_hh1)
    h0f = h0s.reshape(-1, H).astype(np.float64)
    h1f = h1s.reshape(-1, H).astype(np.float64)
    Hh0 = h0f.T @ h0f / len(h0f)
    Hh1 = h1f.T @ h1f / len(h1f)
    Wq_hh0 = _gptq(W_hh0, Hh0)
    Wq_ih1 = _gptq(W_ih1, Hh0)
    Wq_hh1 = _gptq(W_hh1, Hh1)

    # ---- biases (brz0, bhn0, bhn1 pre-scaled by WSCALE) ----
    bias_arr = np.zeros((128, 34), np.float32)
    brz0 = (b_ih0 + b_hh0)[:2 * H]
    brz1 = (b_ih1 + b_hh1)[:2 * H]
    for m in range(8):
        bias_arr[:, m] = brz0[m * 128:(m + 1) * 128]
        bias_arr[:, 16 + m] = brz1[m * 128:(m + 1) * 128]
    for i in range(4):
        bias_arr[:, 8 + i] = b_hh0[2 * H + i * 128:2 * H + (i + 1) * 128] * WSCALE
        bias_arr[:, 12 + i] = b_ih0[2 * H + i * 128:2 * H + (i + 1) * 128]
        bias_arr[:, 24 + i] = b_hh1[2 * H + i * 128:2 * H + (i + 1) * 128] * WSCALE
        bias_arr[:, 28 + i] = b_ih1[2 * H + i * 128:2 * H + (i + 1) * 128]
    bias_arr[:, 32] = b_out[:128]
    bias_arr[:Y1, 33] = b_out[128:Y]

    # ---- bf16 wall: pre-scaled W_ih0 + natural W_out ----
    wall = np.zeros((128, NWALL_BF), np.float32)
    wih0T = (W_ih0 * WSCALE).T            # [Y, 3H]
    wall[:, OFF_WIH0A:OFF_WIH0A + H3] = wih0T[:128]
    wall[:Y1, OFF_WIH0B:OFF_WIH0B + H3] = wih0T[128:Y]
    woutT = W_out.T                        # [H, Y]
    for kc in range(HC):
        wall[:, OFF_WOUT + kc * Y:OFF_WOUT + (kc + 1) * Y] = \
            woutT[kc * 128:(kc + 1) * 128]
    wall[:, OFF_ID:OFF_ID + 128] = np.eye(128, dtype=np.float32)

    # ---- fp8 wall [128, 12, H3] ----
    w8 = np.zeros((128, 12, H3), np.float32)
    for Wq, base in ((Wq_hh0, W8_HH0), (Wq_ih1, W8_IH1), (Wq_hh1, W8_HH1)):
        wT = Wq.T * WSCALE                 # [H, 3H], on the e4m3 grid exactly
        for kc in range(HC):
            w8[:, base + kc, :] = wT[kc * 128:(kc + 1) * 128]
    w8 = np.clip(w8, -240.0, 240.0).astype(ml_dtypes.float8_e4m3)

    bi = np.zeros((128, 34 + 128), np.float32)
    bi[:, 0:34] = bias_arr
    bi[:, 34:34 + 128] = np.eye(128, dtype=np.float32)
    base = {"wall": wall.astype(ml_dtypes.bfloat16), "w8": w8, "bi": bi}
    in_maps = []
    for c in range(NCORE):
        m = dict(base)
        m["xt"] = np.ascontiguousarray(
            x[c * BS:(c + 1) * BS].transpose(2, 1, 0))
        in_maps.append(m)
    return in_maps


def kernel(**inputs):
    global _last_in_maps
    from concourse.bass_utils import run_bass_kernel_spmd
    if "nc" not in _CACHE:
        _CACHE["nc"] = _build()
    in_maps = _prep_in_maps(inputs)
    _last_in_maps = in_maps
    res = run_bass_kernel_spmd(_CACHE["nc"], in_maps, list(range(NCORE)))
    outs = [np.asarray(res.results[i]["out"]).transpose(2, 0, 1)
            for i in range(NCORE)]
    return np.concatenate(outs, axis=0).astype(np.float32)


# revision 23
# speedup vs baseline: 1.0122x; 1.0122x over previous
"""Trainium2 Bass kernel: autoregressive 2-layer GRU decoder (13 steps).

Strategy (per core, batch-sharded 1024/8 = 128):
- Everything lives transposed on-chip: [feature -> partitions, batch -> free].
  GRU biases become per-partition scalars (fused into ScalarE activation /
  DVE scalar_tensor_tensor), and no transposes are needed in the recurrence.
- The 13 autoregressive decode chains are batched along a diagonal wavefront:
  at wavefront step tau, chain k processes window position j = tau - k.  All
  active chains at a given tau read the SAME window element (x_tau or
  y_{tau-13}); it is replicated across the chain block once per tau and its
  projection rides the gate matmul accumulation as extra K-chunks.
- The three big weight matrices (W_hh0, W_ih1, W_hh1; 97% of MACs) are
  GPTQ-quantized to fp8 e4m3 (x4096, clipped to TRN's +-240) against the
  actual h0/h1 activation covariance, and their matmuls run in DoubleRow
  perf mode (K=256/instruction, 2 MACs/cell/cycle).  Hidden states are
  stored as e4m3 at natural scale.  The 2^12 weight scale is folded into
  the ScalarE activation `scale` and pre-scaled gate biases.  W_ih0 (also
  pre-scaled) and W_out stay bf16.
"""
import numpy as np

B, T, Y, H = 1024, 13, 188, 512
H3 = 3 * H
NCORE = 8
BS = B // NCORE          # 128 batch rows per core
HC = H // 128            # 4 H chunks
GC = H3 // 128           # 12 gate chunks
Y1 = Y - 128             # 60 rows in second Y chunk
CPB = 4                  # chains per N-block (4*128 = 512 cols per matmul)

WSCALE = 4096.0          # fp8 weight scale (power of 2)
SINV = 1.0 / WSCALE

GATE_BUFS = 8
TMP_BUFS = 8
EP_BUFS = 8
ER_BUFS = 3
PSA_BUFS = 7

# bf16 wall: W_ih0 (2 K-chunks, pre-scaled x4096) + W_out (4 K-chunks) + I128
OFF_WIH0A = 0
OFF_WIH0B = H3
OFF_WOUT = 2 * H3
OFF_ID = OFF_WOUT + 4 * Y
NWALL_BF = OFF_ID + 128
# fp8 wall [128, 12, H3]: W_hh0 kc 0-3, W_ih1 kc 4-7, W_hh1 kc 8-11
W8_HH0 = 0
W8_IH1 = 4
W8_HH1 = 8

_CACHE = {}
_last_in_maps = None


def _build():
    from contextlib import ExitStack
    import concourse.tile as tile
    from concourse import bacc, mybir

    F32 = mybir.dt.float32
    AF = mybir.ActivationFunctionType
    OP = mybir.AluOpType
    DR = mybir.MatmulPerfMode.DoubleRow

    BF16 = mybir.dt.bfloat16
    F8 = mybir.dt.float8e4
    nc = bacc.Bacc("TRN2", target_bir_lowering=False, debug=False)
    xt = nc.declare_dram_parameter("xt", [Y, T, BS], F32, isOutput=False)
    wall = nc.declare_dram_parameter("wall", [128, NWALL_BF], BF16, isOutput=False)
    w8 = nc.declare_dram_parameter("w8", [128, 12, H3], F8, isOutput=False)
    bi = nc.declare_dram_parameter("bi", [128, 34 + 128], F32, isOutput=False)
    out = nc.declare_dram_parameter("out", [T, Y, BS], F32, isOutput=True)

    with tile.TileContext(nc) as tc, ExitStack() as ctx:
        wp = ctx.enter_context(tc.tile_pool(name="w", bufs=1))
        hp = ctx.enter_context(tc.tile_pool(name="h", bufs=1))
        ep = ctx.enter_context(tc.tile_pool(name="ep", bufs=EP_BUFS))
        ebp = ctx.enter_context(tc.tile_pool(name="ebp", bufs=ER_BUFS))
        xgp = ctx.enter_context(tc.tile_pool(name="xgp", bufs=2))
        gp = ctx.enter_context(tc.tile_pool(name="g", bufs=GATE_BUFS))
        psA = ctx.enter_context(tc.tile_pool(name="psA", bufs=PSA_BUFS, space="PSUM"))
        psO = ctx.enter_context(tc.tile_pool(name="psO", bufs=1, space="PSUM"))

        # ---------------- weights / constants ----------------
        wall_t = wp.tile([128, NWALL_BF], BF16, tag="wall", name="wall")
        nc.sync.dma_start(wall_t[:, :], wall[:, :])
        w8_t = wp.tile([128, 12, H3], F8, tag="w8", name="w8")
        nc.sync.dma_start(w8_t[:, :, :], w8[:, :, :])
        bi_t = wp.tile([128, 34 + 128], F32, tag="bi", name="bi")
        nc.sync.dma_start(bi_t[:, :], bi[:, :])
        wih0_t = [wall_t[:, OFF_WIH0A:OFF_WIH0A + H3],
                  wall_t[:Y1, OFF_WIH0B:OFF_WIH0B + H3]]
        wout_t = [wall_t[:, OFF_WOUT + kc * Y:OFF_WOUT + (kc + 1) * Y]
                  for kc in range(HC)]
        ident_t = wall_t[:, OFF_ID:OFF_ID + 128]
        bias_t = bi_t[:, 0:34]

        # ---------------- persistent state ----------------
        # h stored as fp8 e4m3 (natural scale); a chain's first step (j=0,
        # h=0) skips the recurrent matmuls and uses the h = (1-z)*n update
        hall = hp.tile([128, 2, HC, T, BS], F8, tag="hall", name="hall")
        h0t = hall[:, 0]
        h1t = hall[:, 1]

        # ---------------- helpers ----------------
        def gru_phase(layer, tau, k0, k1, xg0, hlast=None, blocks=None):
            """One wavefront step of a GRU layer over chains k0..k1 inclusive.

            layer 0: shared input projection xg0 [128, GC, BS] (bf16,
            x WSCALE) is broadcast-added across chains on DVE; PSUM holds
            only the recurrent projection.
            layer 1: input projection (from h0) + recurrent projection are
            both accumulated in PSUM.
            The freshest chain (window position j=0) has h=0: its recurrent
            matmuls are skipped and its update is h = (1-z)*n.
            Recurrent/input matmuls are fp8 DoubleRow; psums are x WSCALE.
            For layer 1, the finishing chain's (j=12) h update is written to
            `hlast` in bf16 instead of the fp8 state (feeds W_out directly).
            """
            w8base = W8_HH0 if layer == 0 else W8_HH1
            h = h0t if layer == 0 else h1t
            brz_c = 0 if layer == 0 else 16
            bhn_c = 8 if layer == 0 else 24
            bin_c = 12 if layer == 0 else 28
            fresh_k = tau if layer == 0 else tau - 1   # chain at j=0, if any
            A = k1 - k0 + 1
            if blocks is None:
                blocks = range(0, A, CPB)
            for b0 in blocks:
                ch = min(CPB, A - b0)
                ks = k0 + b0
                # chains with j>0 in this block (fresh chain is always k1,
                # the last chain of the last block)
                cr = ch - 1 if ks + ch - 1 == fresh_k else ch
                # layer-1 chain at its last position, written to hlast
                fin = layer == 1 and hlast is not None and b0 == 0

                def in_proj(ps, m, first, stop):
                    # layer-1 input projection K-pairs into psum ps
                    for j in range(2):
                        nc.tensor.matmul(
                            ps,
                            w8_t[:, W8_IH1 + 2 * j:W8_IH1 + 2 * j + 2,
                                 m * 128:(m + 1) * 128],
                            h0t[:, 2 * j:2 * j + 2, ks:ks + ch, :],
                            start=(first and j == 0),
                            stop=(stop and j == 1), perf_mode=DR)

                def rec_proj(ps, m, first, stop):
                    # recurrent K-pairs, only over the cr non-fresh chains
                    for j in range(2):
                        nc.tensor.matmul(
                            ps,
                            w8_t[:, w8base + 2 * j:w8base + 2 * j + 2,
                                 m * 128:(m + 1) * 128],
                            h[:, 2 * j:2 * j + 2, ks:ks + cr, :],
                            start=(first and j == 0),
                            stop=(stop and j == 1), perf_mode=DR)

                def bc(m, n):
                    return xg0[:, m:m + 1, :].broadcast_to((128, n, BS))

                rzt = {}
                nts = []
                for m in range(8):  # r (0-3) and z (4-7) gates
                    g = gp.tile([128, CPB, BS], BF16, tag="rz", name="rz",
                                bufs=GATE_BUFS)
                    if layer == 0:
                        # psum = rec (fp8 DoubleRow) + xg0 broadcast via an
                        # identity bf16 matmul; fresh chain gets xg0 only
                        # (overwrite where has_written is unset)
                        ps = psA.tile([128, CPB, BS], F32, tag="ps",
                                      name="ps")
                        if cr > 0:
                            rec_proj(ps[:, :cr, :], m, True, False)
                        nc.tensor.matmul(ps[:, :ch, :], ident_t, bc(m, ch),
                                         start=(cr == 0), stop=True,
                                         skip_group_check=True)
                        nc.scalar.activation(g[:, :ch, :], ps[:, :ch, :],
                                             AF.Sigmoid,
                                             bias=bias_t[:, brz_c + m:brz_c + m + 1],
                                             scale=SINV)
                    else:
                        ps = psA.tile([128, CPB, BS], F32, tag="ps", name="ps")
                        if cr > 0:
                            rec_proj(ps[:, :cr, :], m, True, False)
                        in_proj(ps[:, :ch, :], m, cr == 0, True)
                        nc.scalar.activation(g[:, :ch, :], ps[:, :ch, :],
                                             AF.Sigmoid,
                                             bias=bias_t[:, brz_c + m:brz_c + m + 1],
                                             scale=SINV)
                    rzt[m] = g
                for i in range(4):  # n gate
                    m = 8 + i
                    tmp = gp.tile([128, CPB, BS], BF16, tag="tmp", name="tmp",
                                  bufs=TMP_BUFS)
                    if cr > 0:
                        ghn = psA.tile([128, CPB, BS], F32, tag="ps",
                                       name="ghn")
                        rec_proj(ghn[:, :cr, :], m, True, True)
                        nc.vector.scalar_tensor_tensor(
                            tmp[:, :cr, :], ghn[:, :cr, :],
                            bias_t[:, bhn_c + i:bhn_c + i + 1],
                            rzt[i][:, :cr, :], OP.add, OP.mult)
                    if cr < ch:  # fresh chain: gh = 0, tmp = b_hn * r
                        nc.vector.tensor_scalar(
                            tmp[:, cr, :], rzt[i][:, cr, :],
                            bias_t[:, bhn_c + i:bhn_c + i + 1], None, OP.mult)
                    nt = gp.tile([128, CPB, BS], BF16, tag="n", name="n",
                                 bufs=GATE_BUFS)
                    if layer == 0:
                        nc.vector.tensor_add(nt[:, :ch, :], tmp[:, :ch, :],
                                             bc(m, ch))
                    else:
                        xgn = psA.tile([128, CPB, BS], F32, tag="ps",
                                       name="xgn")
                        in_proj(xgn[:, :ch, :], m, True, True)
                        nc.vector.tensor_add(nt[:, :ch, :], tmp[:, :ch, :],
                                             xgn[:, :ch, :])
                    nc.scalar.activation(nt[:, :ch, :], nt[:, :ch, :], AF.Tanh,
                                         bias=bias_t[:, bin_c + i:bin_c + i + 1],
                                         scale=SINV)
                    nts.append(nt)
                for i in range(4):  # h <- n + z*(h - n);  fresh: h <- n - z*n
                    z = rzt[4 + i]
                    d = gp.tile([128, CPB, BS], BF16, tag="tmp", name="d",
                                bufs=TMP_BUFS)
                    if cr > 0:
                        hsl = h[:, i, ks:ks + cr, :]
                        nc.vector.tensor_sub(d[:, :cr, :], hsl,
                                             nts[i][:, :cr, :])
                        nc.vector.tensor_mul(d[:, :cr, :], d[:, :cr, :],
                                             z[:, :cr, :])
                        if fin:
                            # finishing chain -> bf16 staging for W_out
                            nc.vector.tensor_add(hlast[:, i, :], d[:, 0, :],
                                                 nts[i][:, 0, :])
                            if cr > 1:
                                nc.vector.tensor_add(
                                    h[:, i, ks + 1:ks + cr, :],
                                    d[:, 1:cr, :], nts[i][:, 1:cr, :])
                        else:
                            nc.vector.tensor_add(hsl, d[:, :cr, :],
                                                 nts[i][:, :cr, :])
                    if cr < ch:
                        nc.vector.tensor_mul(d[:, cr, :], z[:, cr, :],
                                             nts[i][:, cr, :])
                        nc.vector.tensor_sub(h[:, i, ks + cr, :],
                                             nts[i][:, cr, :], d[:, cr, :])

        def eproj(e_tile, ebf=None):
            """Shared input projection: e [Y, BS] -> xg0 [3H(12 chunks), BS].

            Casts the f32 window element to bf16 (unless out_phase already
            produced it), then 24 N=128 matmuls.
            W_ih0 is pre-scaled by WSCALE, so xg0 is in the scaled domain.
            """
            if ebf is None:
                ebf = ebp.tile([128, 2, BS], BF16, tag="ebf", name="ebf",
                               bufs=ER_BUFS)
                nc.vector.tensor_copy(ebf[:, 0, :], e_tile[:, 0, :])
                nc.vector.tensor_copy(ebf[:Y1, 1, :], e_tile[:Y1, 1, :])
            xg0 = xgp.tile([128, GC, BS], BF16, tag="xg0", name="xg0")
            for third in range(3):
                pe = psO.tile([128, 4, BS], F32, tag="pe", name="pe")
                for mi in range(4):
                    m = third * 4 + mi
                    nc.tensor.matmul(pe[:, mi, :],
                                     wih0_t[0][:, m * 128:(m + 1) * 128],
                                     ebf[:, 0, :], start=True, stop=False)
                    nc.tensor.matmul(pe[:, mi, :],
                                     wih0_t[1][:, m * 128:(m + 1) * 128],
                                     ebf[:Y1, 1, :], start=False, stop=True)
                if third == 1:
                    nc.vector.tensor_copy(
                        xg0[:, third * 4:(third + 1) * 4, :], pe[:, :, :])
                else:
                    nc.scalar.copy(xg0[:, third * 4:(third + 1) * 4, :],
                                   pe[:, :, :])
            return xg0

        def out_phase(tau, elems_bf, hlast):
            """Emit y_{tau-13} = W_out @ relu(h1) + b_out + residual.

            The bf16 residual (previous window element) is accumulated into
            the W_out psum via an identity matmul, and b_out rides the
            ScalarE copy bias, so no DVE op sits on the feedback path."""
            c = tau - 13
            rl = gp.tile([128, HC, BS], BF16, tag="rl", name="rl", bufs=2)
            nc.scalar.activation(rl[:, :, :], hlast[:, :, :], AF.Relu)
            po = psO.tile([128, 4, BS], F32, tag="pe", name="po")[:, :2, :]
            ebr = elems_bf[c + 12]
            for kc in range(HC):
                nc.tensor.matmul(po[:, 0, :], wout_t[kc][:, 0:128],
                                 rl[:, kc, :],
                                 start=(kc == 0), stop=False)
            nc.tensor.matmul(po[:, 0, :], ident_t, ebr[:, 0, :],
                             start=False, stop=True, skip_group_check=True)
            for kc in range(HC):
                nc.tensor.matmul(po[:Y1, 1, :], wout_t[kc][:, 128:Y],
                                 rl[:, kc, :],
                                 start=(kc == 0), stop=False)
            nc.tensor.matmul(po[:Y1, 1, :], ident_t[:Y1, 0:Y1],
                             ebr[:Y1, 1, :],
                             start=False, stop=True, skip_group_check=True)
            ebf = ebp.tile([128, 2, BS], BF16, tag="ebf", name="ebf",
                           bufs=ER_BUFS)
            # bf16 copy first: it heads the y -> xg0 -> L0 feedback path
            nc.scalar.add(ebf[:, 0, :], po[:, 0, :], bias_t[:, 32:33])
            nc.scalar.add(ebf[:Y1, 1, :], po[:Y1, 1, :], bias_t[:Y1, 33:34])
            y = ep.tile([128, 2, BS], F32, tag="e", name="y", bufs=EP_BUFS)
            nc.scalar.add(y[:, 0, :], po[:, 0, :], bias_t[:, 32:33])
            nc.scalar.add(y[:Y1, 1, :], po[:Y1, 1, :], bias_t[:Y1, 33:34])
            # output stays transposed [feature, batch]; host un-transposes
            nc.sync.dma_start(out[c, 0:128, :], y[:, 0, :])
            nc.sync.dma_start(out[c, 128:Y, :], y[:Y1, 1, :])
            return ebf

        # ---------------- wavefront ----------------
        # Issue order per tau: feedback-critical prefix first (L1 block 0 ->
        # out -> eproj -> L0 block 0), then bulk blocks interleaved.  The
        # engine queues are FIFO, so this keeps the autoregressive y ->
        # xg0 -> L0 -> L1 -> y loop from queuing behind bulk gate work.
        elems = {}
        elems_bf = {}
        for tau in range(26):
            if tau <= 12:
                et = ep.tile([128, 2, BS], F32, tag="e", name="e",
                             bufs=EP_BUFS)
                nc.sync.dma_start(et[:, 0, :], xt[0:128, tau, :])
                nc.sync.dma_start(et[:Y1, 1, :], xt[128:Y, tau, :])
                elems[tau] = et
            k0_1, k1_1 = max(0, tau - 13), min(T - 1, tau - 1)
            has1 = k0_1 <= k1_1
            A1 = k1_1 - k0_1 + 1 if has1 else 0
            b1list = list(range(0, A1, CPB))
            k0_0, k1_0 = max(0, tau - 12), min(T - 1, tau)
            has0 = tau <= 24
            A0 = k1_0 - k0_0 + 1 if has0 else 0
            b0list = list(range(0, A0, CPB))
            hlast = None
            if tau >= 13:
                hlast = gp.tile([128, HC, BS], BF16, tag="hlast",
                                name="hlast", bufs=2)
            # --- critical prefix ---
            if has1:
                gru_phase(1, tau, k0_1, k1_1, None, hlast=hlast,
                          blocks=b1list[:1])
            ebf = None
            if tau >= 13:
                ebf = out_phase(tau, elems_bf, hlast)
                elems_bf[tau] = ebf
            xg0 = None
            if has0:
                if ebf is None:
                    ebf = ebp.tile([128, 2, BS], BF16, tag="ebf", name="ebf",
                                   bufs=ER_BUFS)
                    nc.vector.tensor_copy(ebf[:, 0, :], elems[tau][:, 0, :])
                    nc.vector.tensor_copy(ebf[:Y1, 1, :],
                                          elems[tau][:Y1, 1, :])
                    elems_bf[tau] = ebf
                xg0 = eproj(elems.get(tau), ebf)
                gru_phase(0, tau, k0_0, k1_0, xg0, blocks=b0list[:1])
            # --- bulk blocks ---
            if has1 and len(b1list) > 1:
                gru_phase(1, tau, k0_1, k1_1, None, hlast=hlast,
                          blocks=b1list[1:])
            if has0 and len(b0list) > 1:
                gru_phase(0, tau, k0_0, k1_0, xg0, blocks=b0list[1:])

    nc.finalize()
    return nc


def _gru_layer_np(x, W_ih, W_hh, b_ih, b_hh):
    """Full-precision numpy GRU layer for GPTQ calibration."""
    Bc, Tc = x.shape[0], x.shape[1]
    xg = np.einsum('btd,gd->btg', x, W_ih) + b_ih
    h = np.zeros((Bc, H), np.float32)
    hs = []
    for t in range(Tc):
        gh = h @ W_hh.T + b_hh
        r = 1.0 / (1.0 + np.exp(-(xg[:, t, :H] + gh[:, :H])))
        z = 1.0 / (1.0 + np.exp(-(xg[:, t, H:2 * H] + gh[:, H:2 * H])))
        n = np.tanh(xg[:, t, 2 * H:] + r * gh[:, 2 * H:])
        h = (1.0 - z) * n + z * h
        hs.append(h)
    return np.stack(hs, axis=1)


def _q8_grid(a):
    """Round f32 to the e4m3*WSCALE grid (TRN clip at +-240), back to f32."""
    import ml_dtypes
    x = np.clip(np.asarray(a, np.float32) * WSCALE, -240.0, 240.0)
    return np.asarray(x, ml_dtypes.float8_e4m3).astype(np.float32) / WSCALE


def _gptq(W, Hmat, damp=0.01):
    """GPTQ-quantize W [rows, K] to the e4m3 grid given Hessian E[x x^T]."""
    Wf = np.array(W, np.float64)
    K = Wf.shape[1]
    Hd = Hmat + np.eye(K) * damp * np.mean(np.diag(Hmat))
    perm = np.argsort(-np.diag(Hd))
    inv_perm = np.argsort(perm)
    Hd = Hd[perm][:, perm]
    Wp = Wf[:, perm]
    Hinv = np.linalg.inv(Hd)
    U = np.linalg.cholesky(Hinv).T        # upper, Hinv = U^T U
    Q = np.zeros_like(Wp)
    for j in range(K):
        w = Wp[:, j]
        q = _q8_grid(w.astype(np.float32)).astype(np.float64)
        Q[:, j] = q
        err = (w - q) / U[j, j]
        if j + 1 < K:
            Wp[:, j + 1:] -= np.outer(err, U[j, j + 1:])
    return Q[:, inv_perm].astype(np.float32)


def _prep_in_maps(inputs):
    import ml_dtypes
    x = np.asarray(inputs["x"], np.float32)
    f = lambda k: np.asarray(inputs[k], np.float32)
    W_ih0, W_hh0 = f("W_ih0"), f("W_hh0")
    W_ih1, W_hh1 = f("W_ih1"), f("W_hh1")
    W_out = f("W_out")
    b_ih0, b_hh0 = f("b_ih0"), f("b_hh0")
    b_ih1, b_hh1 = f("b_ih1"), f("b_hh1")
    b_out = f("b_out")

    # ---- GPTQ calibration on actual activation trajectories ----
    xs = x[:512]
    h0s = _gru_layer_np(xs, W_ih0, W_hh0, b_ih0, b_hh0)
    h1s = _gru_layer_np(h0s, W_ih1, W_hh1, b_ih1, b_hh1)
    h0f = h0s.reshape(-1, H).astype(np.float64)
    h1f = h1s.reshape(-1, H).astype(np.float64)
    Hh0 = h0f.T @ h0f / len(h0f)
    Hh1 = h1f.T @ h1f / len(h1f)
    Wq_hh0 = _gptq(W_hh0, Hh0)
    Wq_ih1 = _gptq(W_ih1, Hh0)
    Wq_hh1 = _gptq(W_hh1, Hh1)

    # ---- biases (brz0, bhn0, bhn1 pre-scaled by WSCALE) ----
    bias_arr = np.zeros((128, 34), np.float32)
    brz0 = (b_ih0 + b_hh0)[:2 * H]
    brz1 = (b_ih1 + b_hh1)[:2 * H]
    for m in range(8):
        bias_arr[:, m] = brz0[m * 128:(m + 1) * 128]
        bias_arr[:, 16 + m] = brz1[m * 128:(m + 1) * 128]
    for i in range(4):
        bias_arr[:, 8 + i] = b_hh0[2 * H + i * 128:2 * H + (i + 1) * 128] * WSCALE
        bias_arr[:, 12 + i] = b_ih0[2 * H + i * 128:2 * H + (i + 1) * 128]
        bias_arr[:, 24 + i] = b_hh1[2 * H + i * 128:2 * H + (i + 1) * 128] * WSCALE
        bias_arr[:, 28 + i] = b_ih1[2 * H + i * 128:2 * H + (i + 1) * 128]
    bias_arr[:, 32] = b_out[:128]
    bias_arr[:Y1, 33] = b_out[128:Y]

    # ---- bf16 wall: pre-scaled W_ih0 + natural W_out ----
    wall = np.zeros((128, NWALL_BF), np.float32)
    wih0T = (W_ih0 * WSCALE).T            # [Y, 3H]
    wall[:, OFF_WIH0A:OFF_WIH0A + H3] = wih0T[:128]
    wall[:Y1, OFF_WIH0B:OFF_WIH0B + H3] = wih0T[128:Y]
    woutT = W_out.T                        # [H, Y]
    for kc in range(HC):
        wall[:, OFF_WOUT + kc * Y:OFF_WOUT + (kc + 1) * Y] = \
            woutT[kc * 128:(kc + 1) * 128]
    wall[:, OFF_ID:OFF_ID + 128] = np.eye(128, dtype=np.float32)

    # ---- fp8 wall [128, 12, H3] ----
    w8 = np.zeros((128, 12, H3), np.float32)
    for Wq, base in ((Wq_hh0, W8_HH0), (Wq_ih1, W8_IH1), (Wq_hh1, W8_HH1)):
        wT = Wq.T * WSCALE                 # [H, 3H], on the e4m3 grid exactly
        for kc in range(HC):
            w8[:, base + kc, :] = wT[kc * 128:(kc + 1) * 128]
    w8 = np.clip(w8, -240.0, 240.0).astype(ml_dtypes.float8_e4m3)

    bi = np.zeros((128, 34 + 128), np.float32)
    bi[:, 0:34] = bias_arr
    bi[:, 34:34 + 128] = np.eye(128, dtype=np.float32)
    base = {"wall": wall.astype(ml_dtypes.bfloat16), "w8": w8, "bi": bi}
    in_maps = []
    for c in range(NCORE):
        m = dict(base)
        m["xt"] = np.ascontiguousarray(
            x[c * BS:(c + 1) * BS].transpose(2, 1, 0))
        in_maps.append(m)
    return in_maps


def kernel(**inputs):
    global _last_in_maps
    from concourse.bass_utils import run_bass_kernel_spmd
    if "nc" not in _CACHE:
        _CACHE["nc"] = _build()
    in_maps = _prep_in_maps(inputs)
    _last_in_maps = in_maps
    res = run_bass_kernel_spmd(_CACHE["nc"], in_maps, list(range(NCORE)))
    outs = [np.asarray(res.results[i]["out"]).transpose(2, 0, 1)
            for i in range(NCORE)]
    return np.concatenate(outs, axis=0).astype(np.float32)
